# revision 1
# baseline (speedup 1.0000x reference)
"""MoE layer (top-2 routing, 8 experts) on 8 TRN2 NeuronCores.

Strategy: expert-parallel. The host computes routing (router matmul, softmax,
top-2, capacity dispatch — cheap integer/index work) and packs each expert's
tokens densely. Core e runs expert e's MLP (fc -> relu^2 -> proj) over its
padded token batch with fp32r matmuls. The host then gathers, weights and
combines the expert outputs.

All tensors are laid out transposed ([feature, token]) so the device kernel
needs no on-chip transposes: both matmuls contract over the partition dim.
"""
import contextlib
import ctypes
import os

import numpy as np

B, T, C, E, H, K = 4, 2048, 1024, 8, 2048, 2
N = B * T
CAP = 2 * N * K // E  # per-expert capacity; overflow tokens are dropped
NBLK = 512            # token block (matmul moving free dim)
P = 128

_NC_CACHE = {}


def _build_nc(RT):
    from concourse import bacc, mybir, tile

    f32 = mybir.dt.float32
    f32r = mybir.dt.float32r
    KC = C // P   # k-tiles for fc (contract over C)
    KH = H // P   # k-tiles for proj (contract over H)
    NB = RT // NBLK

    nc = bacc.Bacc("TRN2", target_bir_lowering=False, debug=False)
    xeT = nc.dram_tensor("xeT", [C, RT], f32, kind="ExternalInput")
    fcwT = nc.dram_tensor("fcwT", [C, H], f32, kind="ExternalInput")
    pjwT = nc.dram_tensor("pjwT", [H, C], f32, kind="ExternalInput")
    outT = nc.dram_tensor("outT", [C, RT], f32, kind="ExternalOutput")

    xeT_r = xeT[:].rearrange("(k p) n -> p k n", p=P).bitcast(f32r)
    fcw_r = fcwT[:].rearrange("(k p) h -> p k h", p=P).bitcast(f32r)
    pjw_r = pjwT[:].rearrange("(k p) c -> p k c", p=P).bitcast(f32r)
    outT_r = outT[:].rearrange("(m p) n -> p m n", p=P)

    with tile.TileContext(nc) as tc:
        with (
            tc.tile_pool(name="wpool", bufs=1) as wpool,
            tc.tile_pool(name="xpool", bufs=1) as xpool,
            tc.tile_pool(name="hpool", bufs=1) as hpool,
            tc.tile_pool(name="tpool", bufs=3) as tpool,
            tc.tile_pool(name="opool", bufs=3) as opool,
            tc.tile_pool(name="pspool", bufs=4, space="PSUM") as pspool,
        ):
            fcw_sb = wpool.tile([P, KC, H], f32r)
            pjw_sb = wpool.tile([P, KH, C], f32r)
            nc.sync.dma_start(out=fcw_sb[:], in_=fcw_r)
            nc.sync.dma_start(out=pjw_sb[:], in_=pjw_r)

            for nb in range(NB):
                ns = slice(nb * NBLK, (nb + 1) * NBLK)
                x_sb = xpool.tile([P, KC, NBLK], f32r, tag="x")
                nc.sync.dma_start(out=x_sb[:], in_=xeT_r[:, :, ns])

                hid = hpool.tile([P, KH, NBLK], f32r, tag="hid")
                for m in range(KH):
                    ps = pspool.tile([P, NBLK], f32, tag="ps")
                    for k in range(KC):
                        nc.tensor.matmul(
                            ps[:],
                            lhsT=fcw_sb[:, k, m * P:(m + 1) * P],
                            rhs=x_sb[:, k, :],
                            start=(k == 0),
                            stop=(k == KC - 1),
                        )
                    tmp = tpool.tile([P, NBLK], f32, tag="tmp")
                    nc.scalar.activation(
                        tmp[:], ps[:], mybir.ActivationFunctionType.Relu
                    )
                    nc.vector.tensor_tensor(
                        out=hid[:, m, :], in0=tmp[:], in1=tmp[:],
                        op=mybir.AluOpType.mult,
                    )

                for m2 in range(KC):
                    ps2 = pspool.tile([P, NBLK], f32, tag="ps")
                    for k2 in range(KH):
                        nc.tensor.matmul(
                            ps2[:],
                            lhsT=pjw_sb[:, k2, m2 * P:(m2 + 1) * P],
                            rhs=hid[:, k2, :],
                            start=(k2 == 0),
                            stop=(k2 == KH - 1),
                        )
                    o_sb = opool.tile([P, NBLK], f32, tag="o")
                    nc.vector.tensor_copy(o_sb[:], ps2[:])
                    nc.sync.dma_start(out=outT_r[:, m2, ns], in_=o_sb[:])

    nc.compile()
    return nc


def _profile_hook():
    """NTFF capture via libaxon ctypes (used only when MOE_PROFILE_DIR is set)."""
    so_path = "/opt/axon/libaxon_pjrt.so"
    if not os.path.exists(so_path):
        return None
    lib = ctypes.CDLL(so_path)
    if not hasattr(lib, "axon_start_nrt_profile"):
        return None
    lib.axon_start_nrt_profile.argtypes = [
        ctypes.POINTER(ctypes.c_int64), ctypes.c_size_t,
    ]
    lib.axon_start_nrt_profile.restype = ctypes.c_int64
    lib.axon_stop_nrt_profile.argtypes = [ctypes.c_char_p]
    lib.axon_stop_nrt_profile.restype = ctypes.c_int64

    @contextlib.contextmanager
    def _hook(output_dir, device_ids):
        import jax
        jax.devices()
        ids = (ctypes.c_int64 * len(device_ids))(*device_ids)
        rc = lib.axon_start_nrt_profile(ids, len(device_ids))
        if rc != 0:
            raise RuntimeError(f"axon_start_nrt_profile rc={rc}")
        try:
            yield
        finally:
            n = lib.axon_stop_nrt_profile(str(output_dir).encode())
            print(f"profile: {n} file(s) written to {output_dir}")

    return _hook


def _run_device(xeT, fcwT_all, pjwT_all, RT):
    from concourse.bass_utils import run_bass_kernel_spmd

    if RT not in _NC_CACHE:
        _NC_CACHE[RT] = _build_nc(RT)
    nc = _NC_CACHE[RT]

    in_maps = [
        {"xeT": xeT[e], "fcwT": fcwT_all[e], "pjwT": pjwT_all[e]}
        for e in range(E)
    ]
    core_ids = list(range(E))

    prof_dir = os.environ.get("MOE_PROFILE_DIR")
    if prof_dir:
        hook = _profile_hook()
        # warm-up run first so the profiled run measures a warm device
        run_bass_kernel_spmd(nc, in_maps, core_ids)
        with hook(prof_dir, core_ids):
            res = run_bass_kernel_spmd(nc, in_maps, core_ids)
    else:
        res = run_bass_kernel_spmd(nc, in_maps, core_ids)

    return np.stack([res.results[e]["outT"] for e in range(E)])


def kernel(x, router_w, fc_w, proj_w):
    x = np.asarray(x, np.float32)
    router_w = np.asarray(router_w, np.float32)
    fc_w = np.asarray(fc_w, np.float32)
    proj_w = np.asarray(proj_w, np.float32)

    x_flat = x.reshape(N, C)

    # --- routing (mirrors the reference numerics in f32) ---
    scores = x_flat @ router_w.T                        # [N, E]
    mx = scores.max(-1, keepdims=True)
    ex = np.exp(scores - mx)
    probs = (ex / ex.sum(-1, keepdims=True)).astype(np.float32)
    top_idx = np.argsort(-probs, axis=-1, kind="stable")[:, :K]
    top_w = np.take_along_axis(probs, top_idx, -1)
    top_w = top_w / (top_w.sum(-1, keepdims=True) + 1e-10)

    fe = top_idx.reshape(-1)                            # [N*K]
    fw = top_w.reshape(-1).astype(np.float32)
    ft = np.repeat(np.arange(N), K)
    order = np.argsort(fe, kind="stable")
    se, st = fe[order], ft[order]
    counts = np.bincount(fe, minlength=E)
    offs = np.concatenate([[0], np.cumsum(counts)[:-1]]).astype(np.int64)
    pos_sorted = np.arange(N * K) - offs[se]
    kept = np.minimum(counts, CAP)

    RT = max(int(np.ceil(kept.max() / NBLK)), 1) * NBLK

    # --- pack per-expert token batches, transposed ---
    xeT = np.zeros((E, C, RT), np.float32)
    for e in range(E):
        toks = st[offs[e]: offs[e] + kept[e]]
        xeT[e, :, :kept[e]] = x_flat[toks].T
    fcwT_all = np.ascontiguousarray(fc_w.transpose(0, 2, 1))   # [E, C, H]
    pjwT_all = np.ascontiguousarray(proj_w.transpose(0, 2, 1)) # [E, H, C]

    outT = _run_device(xeT, fcwT_all, pjwT_all, RT)            # [E, C, RT]

    # --- combine: weight each (token, slot) contribution and sum ---
    inv = np.empty(N * K, np.int64)
    inv[order] = np.arange(N * K)
    pos_flat = pos_sorted[inv]
    valid = pos_flat < CAP
    pos_c = np.where(valid, pos_flat, 0)
    w_eff = np.where(valid, fw, 0.0).astype(np.float32)

    gathered = outT[fe, :, pos_c]                              # [N*K, C]
    out = (gathered * w_eff[:, None]).reshape(N, K, C).sum(1)

    return (
        out.reshape(B, T, C).astype(np.float32),
        probs.reshape(B, T, E).astype(np.float32),
    )


# revision 3
# speedup vs baseline: 1.2219x; 1.2219x over previous
"""MoE layer (top-2 routing, 8 experts) on 8 TRN2 NeuronCores.

Strategy: expert-parallel. The host computes routing (router matmul, softmax,
top-2, capacity dispatch — cheap integer/index work) and packs each expert's
tokens densely. Core e runs expert e's MLP (fc -> relu^2 -> proj) over its
padded token batch with fp32r matmuls. The host then gathers, weights and
combines the expert outputs.

All tensors are laid out transposed ([feature, token]) so the device kernel
needs no on-chip transposes: both matmuls contract over the partition dim.
"""
import contextlib
import ctypes
import os

import numpy as np

B, T, C, E, H, K = 4, 2048, 1024, 8, 2048, 2
N = B * T
CAP = 2 * N * K // E  # per-expert capacity; overflow tokens are dropped
NBLK = 512            # token block (matmul moving free dim)
P = 128

_NC_CACHE = {}


def _block_widths(max_kept):
    """Token-block widths covering max_kept: full 512s plus a 256 tail if it fits."""
    full, rem = divmod(max(int(max_kept), 1), NBLK)
    if rem == 0:
        return [NBLK] * full
    if rem <= NBLK // 2:
        return [NBLK] * full + [NBLK // 2]
    return [NBLK] * (full + 1)


def _build_nc(blocks, mode):
    from concourse import bacc, mybir, tile

    f32 = mybir.dt.float32
    cd = {"f32r": mybir.dt.float32r, "bf16": mybir.dt.bfloat16,
          "f32": mybir.dt.float32}[mode]
    io_dt = f32 if mode in ("f32r", "f32") else mybir.dt.bfloat16
    KC = C // P   # k-tiles for fc (contract over C)
    KH = H // P   # k-tiles for proj (contract over H)
    RT = sum(blocks)

    def rcast(ap):
        return ap.bitcast(cd) if mode == "f32r" else ap

    nc = bacc.Bacc("TRN2", target_bir_lowering=False, debug=False)
    xeT = nc.dram_tensor("xeT", [C, RT], io_dt, kind="ExternalInput")
    fcwT = nc.dram_tensor("fcwT", [C, H], io_dt, kind="ExternalInput")
    pjwT = nc.dram_tensor("pjwT", [H, C], io_dt, kind="ExternalInput")
    outT = nc.dram_tensor("outT", [C, RT], f32, kind="ExternalOutput")

    xeT_r = rcast(xeT[:].rearrange("(k p) n -> p k n", p=P))
    fcw_r = rcast(fcwT[:].rearrange("(k p) h -> p k h", p=P))
    pjw_r = rcast(pjwT[:].rearrange("(k p) c -> p k c", p=P))
    outT_r = outT[:].rearrange("(m p) n -> p m n", p=P)

    with tile.TileContext(nc) as tc:
        with (
            tc.tile_pool(name="wpool", bufs=1) as wpool,
            tc.tile_pool(name="xpool", bufs=2) as xpool,
            tc.tile_pool(name="hpool", bufs=1) as hpool,
            tc.tile_pool(name="tpool", bufs=3) as tpool,
            tc.tile_pool(name="opool", bufs=3) as opool,
            tc.tile_pool(name="pspool", bufs=4, space="PSUM") as pspool,
        ):
            fcw_sb = wpool.tile([P, KC, H], cd)
            pjw_sb = wpool.tile([P, KH, C], cd)
            # Weight loads split per k-chunk so the first matmul only waits
            # for its own chunk; fc on the sync queue, proj on scalar's.
            for k in range(KC):
                nc.sync.dma_start(out=fcw_sb[:, k, :], in_=fcw_r[:, k, :])
            for k in range(KH):
                nc.scalar.dma_start(out=pjw_sb[:, k, :], in_=pjw_r[:, k, :])

            col = 0
            for nb, width in enumerate(blocks):
                ns = slice(col, col + width)
                col += width
                x_sb = xpool.tile([P, KC, NBLK], cd, tag="x")
                if nb == 0:
                    # split so MM1 m=0 k=0 can start after one small chunk
                    for k in range(KC):
                        nc.sync.dma_start(out=x_sb[:, k, :width],
                                          in_=xeT_r[:, k, ns])
                else:
                    nc.gpsimd.dma_start(out=x_sb[:, :, :width],
                                        in_=xeT_r[:, :, ns])

                hid = hpool.tile([P, KH, NBLK], cd, tag="hid")
                for m in range(KH):
                    ps = pspool.tile([P, NBLK], f32, tag="ps")
                    for k in range(KC):
                        nc.tensor.matmul(
                            ps[:, :width],
                            lhsT=fcw_sb[:, k, m * P:(m + 1) * P],
                            rhs=x_sb[:, k, :width],
                            start=(k == 0),
                            stop=(k == KC - 1),
                        )
                    tmp = tpool.tile([P, NBLK], f32, tag="tmp")
                    nc.scalar.activation(
                        tmp[:, :width], ps[:, :width],
                        mybir.ActivationFunctionType.Relu,
                    )
                    nc.vector.tensor_tensor(
                        out=hid[:, m, :width], in0=tmp[:, :width],
                        in1=tmp[:, :width], op=mybir.AluOpType.mult,
                    )

                for m2 in range(KC):
                    ps2 = pspool.tile([P, NBLK], f32, tag="ps")
                    for k2 in range(KH):
                        nc.tensor.matmul(
                            ps2[:, :width],
                            lhsT=pjw_sb[:, k2, m2 * P:(m2 + 1) * P],
                            rhs=hid[:, k2, :width],
                            start=(k2 == 0),
                            stop=(k2 == KH - 1),
                        )
                    o_sb = opool.tile([P, NBLK], f32, tag="o")
                    nc.vector.tensor_copy(o_sb[:, :width], ps2[:, :width])
                    nc.sync.dma_start(out=outT_r[:, m2, ns], in_=o_sb[:, :width])

    nc.compile()
    return nc


def _profile_hook():
    """NTFF capture via libaxon ctypes (used only when MOE_PROFILE_DIR is set)."""
    so_path = "/opt/axon/libaxon_pjrt.so"
    if not os.path.exists(so_path):
        return None
    lib = ctypes.CDLL(so_path)
    if not hasattr(lib, "axon_start_nrt_profile"):
        return None
    lib.axon_start_nrt_profile.argtypes = [
        ctypes.POINTER(ctypes.c_int64), ctypes.c_size_t,
    ]
    lib.axon_start_nrt_profile.restype = ctypes.c_int64
    lib.axon_stop_nrt_profile.argtypes = [ctypes.c_char_p]
    lib.axon_stop_nrt_profile.restype = ctypes.c_int64

    @contextlib.contextmanager
    def _hook(output_dir, device_ids):
        import jax
        jax.devices()
        ids = (ctypes.c_int64 * len(device_ids))(*device_ids)
        rc = lib.axon_start_nrt_profile(ids, len(device_ids))
        if rc != 0:
            raise RuntimeError(f"axon_start_nrt_profile rc={rc}")
        try:
            yield
        finally:
            n = lib.axon_stop_nrt_profile(str(output_dir).encode())
            print(f"profile: {n} file(s) written to {output_dir}")

    return _hook


def _run_device(xeT, fcwT_all, pjwT_all, blocks, mode):
    from concourse.bass_utils import run_bass_kernel_spmd

    key = (tuple(blocks), mode)
    if key not in _NC_CACHE:
        _NC_CACHE[key] = _build_nc(blocks, mode)
    nc = _NC_CACHE[key]

    in_maps = [
        {"xeT": xeT[e], "fcwT": fcwT_all[e], "pjwT": pjwT_all[e]}
        for e in range(E)
    ]
    core_ids = list(range(E))

    prof_dir = os.environ.get("MOE_PROFILE_DIR")
    if prof_dir:
        hook = _profile_hook()
        # warm-up run first so the profiled run measures a warm device
        run_bass_kernel_spmd(nc, in_maps, core_ids)
        with hook(prof_dir, core_ids):
            res = run_bass_kernel_spmd(nc, in_maps, core_ids)
    else:
        res = run_bass_kernel_spmd(nc, in_maps, core_ids)

    return np.stack([res.results[e]["outT"] for e in range(E)])


def kernel(x, router_w, fc_w, proj_w):
    x = np.asarray(x, np.float32)
    router_w = np.asarray(router_w, np.float32)
    fc_w = np.asarray(fc_w, np.float32)
    proj_w = np.asarray(proj_w, np.float32)

    mode = os.environ.get("MOE_DTYPE", "f32r")
    x_flat = x.reshape(N, C)

    # --- routing (mirrors the reference numerics in f32) ---
    scores = x_flat @ router_w.T                        # [N, E]
    mx = scores.max(-1, keepdims=True)
    ex = np.exp(scores - mx)
    probs = (ex / ex.sum(-1, keepdims=True)).astype(np.float32)
    top_idx = np.argsort(-probs, axis=-1, kind="stable")[:, :K]
    top_w = np.take_along_axis(probs, top_idx, -1)
    top_w = top_w / (top_w.sum(-1, keepdims=True) + 1e-10)

    fe = top_idx.reshape(-1)                            # [N*K]
    fw = top_w.reshape(-1).astype(np.float32)
    ft = np.repeat(np.arange(N), K)
    order = np.argsort(fe, kind="stable")
    se, st = fe[order], ft[order]
    counts = np.bincount(fe, minlength=E)
    offs = np.concatenate([[0], np.cumsum(counts)[:-1]]).astype(np.int64)
    pos_sorted = np.arange(N * K) - offs[se]
    kept = np.minimum(counts, CAP)

    blocks = _block_widths(kept.max())
    RT = sum(blocks)

    if mode == "bf16":
        import ml_dtypes
        io_np = ml_dtypes.bfloat16
    else:
        io_np = np.float32

    # --- pack per-expert token batches, transposed ---
    xeT = np.zeros((E, C, RT), io_np)
    for e in range(E):
        toks = st[offs[e]: offs[e] + kept[e]]
        xeT[e, :, :kept[e]] = x_flat[toks].T.astype(io_np)
    fcwT_all = np.ascontiguousarray(
        fc_w.transpose(0, 2, 1)).astype(io_np)    # [E, C, H]
    pjwT_all = np.ascontiguousarray(
        proj_w.transpose(0, 2, 1)).astype(io_np)  # [E, H, C]

    outT = _run_device(xeT, fcwT_all, pjwT_all, blocks, mode)  # [E, C, RT]

    # --- combine: weight each (token, slot) contribution and sum ---
    inv = np.empty(N * K, np.int64)
    inv[order] = np.arange(N * K)
    pos_flat = pos_sorted[inv]
    valid = pos_flat < CAP
    pos_c = np.where(valid, pos_flat, 0)
    w_eff = np.where(valid, fw, 0.0).astype(np.float32)

    gathered = outT[fe, :, pos_c]                              # [N*K, C]
    out = (gathered * w_eff[:, None]).reshape(N, K, C).sum(1)

    return (
        out.reshape(B, T, C).astype(np.float32),
        probs.reshape(B, T, E).astype(np.float32),
    )
